# Initial kernel scaffold
#
"""Trainium2 Bass kernel for nn_Attention (sparse_attention).

Math (reference collapsed):
  va[b]    = ht[b] @ Wa_w          (host, tiny)
  ca[b]    = ht[b] . Wa_b          (host, tiny)
  energy   = leaky_relu(hs . va + ca), masked to -1e4 where s >= state_len
  alpha    = softmax(energy)                               (device)
  u[b]     = sum_s alpha[b,s] * hs[b,s,:]                  (device)
  context  = u @ Wc_w.T + Wc_b                             (host, tiny)

Device strategy: pure data-parallel over batch (4 batches/core x 8 cores).
hs is shipped twice in fp16 (natural layout for the alpha-weighted sum,
pre-transposed layout for the energy matvec) so both matmuls contract over
the SBUF partition axis. All heavy traffic is 8 MiB/batch of HBM reads.
"""

import sys

sys.path.insert(0, "/opt/trn_rl_repo")

import numpy as np

import concourse.bass as bass
import concourse.tile as tile
from concourse import bass_isa, mybir
from concourse.bass_utils import run_bass_kernel_spmd
from concourse.vector_clock import ScopedClock

B, S, E = 32, 8192, 256
NCORES = 8
BL = B // NCORES  # batches per core
NT = S // 128  # 64 s-tiles of 128
NEG_SLOPE = 0.2
MASK_VAL = -10000.0
AF = mybir.ActivationFunctionType
ALU = mybir.AluOpType

_PATCHED = False


def _patch_tile_drain():
    """Walrus in this env rejects >1 sem-wait on the kernel-exit Drain CTRL.

    Hoist the end-of-kernel waits onto single-wait sync nops instead.
    """
    global _PATCHED
    if _PATCHED:
        return
    _PATCHED = True

    def _drain_and_barrier(self, tick_clock, wait_clock):
        nc = self.nc
        carrier = nc.sync.nop(nofuse=True, hint="tile_exit_wait_carrier")
        wait_clock.add_sem_waits(
            carrier.ins, ScopedClock({None: tick_clock.global_clock})
        )
        si = carrier.ins.sync_info
        waits = list(si.on_wait) if si is not None else []
        if len(waits) > 1:
            carrier.ins.sync_info = mybir.SyncInfo(
                on_wait=[waits[0]], on_update=list(si.on_update)
            )
            for w in waits[1:]:
                n2 = nc.sync.nop(nofuse=True, hint="tile_exit_wait")
                n2.ins.sync_info = mybir.SyncInfo(on_wait=[w], on_update=[])
        nc.sync.drain(fusable=False)
        nc.all_engine_barrier()
        assert self.sems is not None
        popped = nc._tile_sem_poison_stack.pop()
        assert popped is self._sem_poison
        nc.clear_and_free_semaphores(list(self.sems.allocated().values()))
        nc.all_engine_barrier()

    tile.TileContext._drain_and_barrier = _drain_and_barrier


def _build_bass():
    _patch_tile_drain()
    nc = bass.Bass("TRN2", target_bir_lowering=False, num_devices=NCORES)
    f16, f32 = mybir.dt.float16, mybir.dt.float32

    hs16 = nc.declare_dram_parameter("hs16", [BL, S, E], f16)
    hsT16 = nc.declare_dram_parameter("hsT16", [BL, E, S], f16)
    va16 = nc.declare_dram_parameter("va16", [BL, 128, 2], f16)
    # aux[:, :, :NT] = mask cap (+BIG valid / -1e4 invalid), aux[:, :, NT] = ca
    aux = nc.declare_dram_parameter("aux", [BL, 128, NT + 1], f32)
    alpha_o = nc.declare_dram_parameter("alpha", [BL, 128, NT], f32, isOutput=True)
    u_o = nc.declare_dram_parameter("u", [BL, 1, E], f32, isOutput=True)

    with tile.TileContext(nc) as tc:
        with (
            tc.tile_pool(name="big", bufs=2) as big,
            tc.tile_pool(name="small", bufs=2) as small,
            tc.tile_pool(name="pse", bufs=2, space="PSUM") as pse,
            tc.tile_pool(name="psu", bufs=2, space="PSUM") as psu,
        ):
            for b in range(BL):
                # ---- loads ----
                hsT = big.tile([128, 2 * S], f16, tag="hsT")
                nc.sync.dma_start(out=hsT[:, 0:S], in_=hsT16[b, 0:128, :])
                nc.sync.dma_start(out=hsT[:, S : 2 * S], in_=hsT16[b, 128:256, :])
                hsn = big.tile([128, NT, E], f16, tag="hsn")
                nc.sync.dma_start(
                    out=hsn[:], in_=hs16[b].rearrange("(t p) e -> p t e", p=128)
                )
                va = small.tile([128, 2], f16, tag="va")
                nc.sync.dma_start(out=va[:], in_=va16[b])
                ax = small.tile([128, NT + 1], f32, tag="aux")
                nc.sync.dma_start(out=ax[:], in_=aux[b])

                # ---- energy: e[t*128+p] = sum_e hs[s,e]*va[e]  (PE) ----
                pe = pse.tile([128, NT], f32, tag="pe")
                for t in range(NT):
                    for h in range(2):
                        nc.tensor.matmul(
                            out=pe[:, t : t + 1],
                            lhsT=hsT[:, h * S + t * 128 : h * S + (t + 1) * 128],
                            rhs=va[:, h : h + 1],
                            start=(h == 0),
                            stop=(h == 1),
                        )

                # ---- epilogue: lrelu(energy + ca) ----
                esb = small.tile([128, NT], f32, tag="esb")
                nc.scalar.activation(
                    out=esb[:],
                    in_=pe[:],
                    func=AF.Lrelu,
                    bias=ax[:, NT : NT + 1],
                    scale=1.0,
                    alpha=NEG_SLOPE,
                )
                # ---- mask via min(e, cap), fused with row-max ----
                em = small.tile([128, NT], f32, tag="em")
                mrow = small.tile([128, 1], f32, tag="mrow")
                nc.vector.tensor_tensor_reduce(
                    out=em[:],
                    in0=esb[:],
                    in1=ax[:, 0:NT],
                    scale=1.0,
                    scalar=-3.0e38,
                    op0=ALU.min,
                    op1=ALU.max,
                    accum_out=mrow[:],
                )
                # ---- global max / exp / global sum ----
                mall = small.tile([128, 1], f32, tag="mall")
                nc.gpsimd.partition_all_reduce(
                    mall[:], mrow[:], channels=128, reduce_op=bass_isa.ReduceOp.max
                )
                negm = small.tile([128, 1], f32, tag="negm")
                nc.vector.tensor_scalar_mul(negm[:], mall[:], -1.0)
                p16 = small.tile([128, NT], f16, tag="p16")
                zrow = small.tile([128, 1], f32, tag="zrow")
                nc.scalar.activation(
                    out=p16[:],
                    in_=em[:],
                    func=AF.Exp,
                    bias=negm[:],
                    scale=1.0,
                    accum_out=zrow[:],
                )
                zall = small.tile([128, 1], f32, tag="zall")
                nc.gpsimd.partition_all_reduce(
                    zall[:], zrow[:], channels=128, reduce_op=bass_isa.ReduceOp.add
                )
                rz = small.tile([128, 1], f32, tag="rz")
                nc.vector.reciprocal(rz[:], zall[:])

                # ---- alpha output ----
                a32 = small.tile([128, NT], f32, tag="a32")
                nc.vector.tensor_scalar_mul(a32[:], p16[:], rz[:])
                nc.sync.dma_start(out=alpha_o[b], in_=a32[:])

                # ---- u = sum_s p[s]*hs[s,:] (PE, accumulated), then /z ----
                pu = psu.tile([1, E], f32, tag="pu")
                for t in range(NT):
                    nc.tensor.matmul(
                        out=pu[:],
                        lhsT=p16[:, t : t + 1],
                        rhs=hsn[:, t, :],
                        start=(t == 0),
                        stop=(t == NT - 1),
                    )
                usb = small.tile([1, E], f32, tag="usb")
                nc.scalar.activation(
                    out=usb[:], in_=pu[:], func=AF.Copy, scale=rz[0:1, :]
                )
                nc.sync.dma_start(out=u_o[b], in_=usb[:])
    return nc


_NC_CACHE = None
last_results = None  # exposes BassKernelResults (exec_time_ns etc.) to test.py


def kernel(hs, ht, state_len, Wa_w, Wa_b, Wc_w, Wc_b, **run_kwargs):
    global _NC_CACHE, last_results
    hs = np.asarray(hs, np.float32)
    ht = np.asarray(ht, np.float32)
    state_len = np.asarray(state_len, np.int32)
    Wa_w = np.asarray(Wa_w, np.float32)
    Wa_b = np.asarray(Wa_b, np.float32)
    Wc_w = np.asarray(Wc_w, np.float32)
    Wc_b = np.asarray(Wc_b, np.float32)

    # host-side tiny precomputes
    va = ht @ Wa_w  # [B, E]
    ca = ht @ Wa_b  # [B]
    hs16 = hs.astype(np.float16)
    hsT16 = np.ascontiguousarray(hs16.transpose(0, 2, 1))  # [B, E, S]
    va16_dev = np.ascontiguousarray(
        va.astype(np.float16).reshape(B, 2, 128).transpose(0, 2, 1)
    )  # [B, 128, 2]; va16_dev[b, p, h] = va[b, h*128+p]
    pos = np.arange(S, dtype=np.int64)
    valid = pos[None, :] < state_len[:, None].astype(np.int64)  # [B, S]
    cap = np.where(valid, 3.0e38, MASK_VAL).astype(np.float32)  # [B, S]
    cap_dev = cap.reshape(B, NT, 128).transpose(0, 2, 1)  # [B, 128, NT]
    aux = np.concatenate(
        [
            cap_dev,
            np.broadcast_to(ca.astype(np.float32)[:, None, None], (B, 128, 1)),
        ],
        axis=2,
    )
    aux = np.ascontiguousarray(aux, np.float32)

    if _NC_CACHE is None:
        _NC_CACHE = _build_bass()
    nc = _NC_CACHE

    in_maps = []
    for i in range(NCORES):
        sl = slice(i * BL, (i + 1) * BL)
        in_maps.append(
            {
                "hs16": hs16[sl],
                "hsT16": hsT16[sl],
                "va16": va16_dev[sl],
                "aux": aux[sl],
            }
        )
    res = run_bass_kernel_spmd(nc, in_maps, core_ids=list(range(NCORES)), **run_kwargs)
    last_results = res

    alpha_dev = np.stack([res.results[i]["alpha"] for i in range(NCORES)])
    # alpha_dev[core, b, p, t] -> s = t*128 + p
    alpha = (
        alpha_dev.transpose(0, 1, 3, 2).reshape(B, S).astype(np.float32)
    )
    u = np.concatenate(
        [res.results[i]["u"].reshape(BL, E) for i in range(NCORES)]
    )  # [B, E]
    context = (u @ Wc_w.T + Wc_b).astype(np.float32)
    return alpha, context


# revision 11
# speedup vs baseline: 1.0682x; 1.0682x over previous
"""Trainium2 Bass kernel for nn_Attention (sparse_attention).

Math (reference collapsed):
  va[b]    = ht[b] @ Wa_w          (host, tiny)
  ca[b]    = ht[b] . Wa_b          (host, tiny)
  energy   = leaky_relu(hs . va + ca), masked to -1e4 where s >= state_len
  alpha    = softmax(energy)                               (device)
  u[b]     = sum_s alpha[b,s] * hs[b,s,:]                  (device)
  context  = u @ Wc_w.T + Wc_b                             (host, tiny)

Device strategy: pure data-parallel over batch (4 batches/core x 8 cores).
hs is shipped twice in fp16 (natural layout for the alpha-weighted sum,
pre-transposed layout for the energy matvec) so both matmuls contract over
the SBUF partition axis. All heavy traffic is 8 MiB/batch of HBM reads.
"""

import sys

sys.path.insert(0, "/opt/trn_rl_repo")

import numpy as np

import concourse.bass as bass
import concourse.tile as tile
from concourse import bass_isa, mybir
from concourse.bass_utils import run_bass_kernel_spmd
from concourse.vector_clock import ScopedClock

B, S, E = 32, 8192, 256
NCORES = 8
BL = B // NCORES  # batches per core
NT = S // 128  # 64 s-tiles of 128
NEG_SLOPE = 0.2
MASK_VAL = -10000.0
AF = mybir.ActivationFunctionType
ALU = mybir.AluOpType

_PATCHED = False


def _patch_tile_drain():
    """Walrus in this env rejects >1 sem-wait on the kernel-exit Drain CTRL.

    Hoist the end-of-kernel waits onto single-wait sync nops instead.
    """
    global _PATCHED
    if _PATCHED:
        return
    _PATCHED = True

    def _drain_and_barrier(self, tick_clock, wait_clock):
        nc = self.nc
        carrier = nc.sync.nop(nofuse=True, hint="tile_exit_wait_carrier")
        wait_clock.add_sem_waits(
            carrier.ins, ScopedClock({None: tick_clock.global_clock})
        )
        si = carrier.ins.sync_info
        waits = list(si.on_wait) if si is not None else []
        if len(waits) > 1:
            carrier.ins.sync_info = mybir.SyncInfo(
                on_wait=[waits[0]], on_update=list(si.on_update)
            )
            for w in waits[1:]:
                n2 = nc.sync.nop(nofuse=True, hint="tile_exit_wait")
                n2.ins.sync_info = mybir.SyncInfo(on_wait=[w], on_update=[])
        nc.sync.drain(fusable=False)
        nc.all_engine_barrier()
        assert self.sems is not None
        popped = nc._tile_sem_poison_stack.pop()
        assert popped is self._sem_poison
        nc.clear_and_free_semaphores(list(self.sems.allocated().values()))
        nc.all_engine_barrier()

    tile.TileContext._drain_and_barrier = _drain_and_barrier


def _split_sync_waits(nc, max_waits=1):
    """Walrus in this env rejects >N sem-waits on a single instruction.

    Hoist excess waits onto same-engine NoOps placed immediately before the
    instruction — the engine sequencer executes in order, so waiting earlier
    on the same engine is semantically identical.
    """
    counter = 0
    for fn in nc.m.functions:
        for blk in fn.blocks:
            insts = list(blk.instructions)
            out = []
            changed = False
            for inst in insts:
                si = inst.sync_info
                waits = list(si.on_wait) if si is not None else []
                if len(waits) > max_waits:
                    keep = waits[:max_waits]
                    for w in waits[max_waits:]:
                        nop = mybir.InstNoOp(name=f"WSPLIT-{counter}")
                        counter += 1
                        nop.engine = inst.engine
                        nop.sync_info = mybir.SyncInfo(on_wait=[w], on_update=[])
                        out.append(nop)
                    inst.sync_info = mybir.SyncInfo(
                        on_wait=keep, on_update=list(si.on_update)
                    )
                    changed = True
                out.append(inst)
            if changed:
                blk.instructions = out


def _build_bass(reps=1):
    _patch_tile_drain()
    nc = bass.Bass("TRN2", target_bir_lowering=False, num_devices=NCORES)
    f16, f32 = mybir.dt.float16, mybir.dt.float32

    hs16 = nc.declare_dram_parameter("hs16", [BL, S, E], f16, isOutput=False)
    hsT16 = nc.declare_dram_parameter("hsT16", [BL, E, S], f16, isOutput=False)
    va16 = nc.declare_dram_parameter("va16", [BL, 128, 2], f16, isOutput=False)
    # aux[:, :, :NT] = mask cap (+BIG valid / -1e4 invalid), aux[:, :, NT] = ca
    aux = nc.declare_dram_parameter("aux", [BL, 128, NT + 1], f32, isOutput=False)
    # consts[:, 0:128] = eye(128), consts[:, 128:256] = all-ones
    consts = nc.declare_dram_parameter("consts", [128, 256], f32, isOutput=False)
    alpha_o = nc.declare_dram_parameter("alpha", [BL, 128, NT], f32, isOutput=True)
    u_o = nc.declare_dram_parameter("u", [BL, 1, E], f32, isOutput=True)

    with tile.TileContext(nc) as tc:
        with (
            tc.tile_pool(name="big", bufs=2) as big,
            tc.tile_pool(name="small", bufs=2) as small,
            tc.tile_pool(name="ones", bufs=1) as onesp,
            tc.tile_pool(name="pse", bufs=2, space="PSUM") as pse,
            tc.tile_pool(name="psu", bufs=2, space="PSUM") as psu,
            tc.tile_pool(name="pss", bufs=4, space="PSUM") as pss,
        ):
            cst = onesp.tile([128, 256], f32)
            nc.sync.dma_start(out=cst[:], in_=consts[:])
            ident = cst[:, 0:128]
            onescol = cst[:, 128:129]
            onesrow = cst[0:1, 128:256]
            for b in [b for _ in range(reps) for b in range(BL)]:
                # ---- loads ----
                hsT = big.tile([128, 2 * S], f16, tag="hsT")
                nc.sync.dma_start(out=hsT[:, 0:S], in_=hsT16[b, 0:128, :])
                nc.sync.dma_start(out=hsT[:, S : 2 * S], in_=hsT16[b, 128:256, :])
                hsn = big.tile([128, NT, E], f16, tag="hsn")
                nc.sync.dma_start(
                    out=hsn[:], in_=hs16[b].rearrange("(t p) e -> p t e", p=128)
                )
                va = small.tile([128, 2], f16, tag="va")
                nc.sync.dma_start(out=va[:], in_=va16[b])
                ax = small.tile([128, NT + 1], f32, tag="aux")
                nc.sync.dma_start(out=ax[:], in_=aux[b])

                # ---- energy: e[t*128+p] = sum_e hs[s,e]*va[e]  (PE) ----
                pe = pse.tile([128, NT], f32, tag="pe")
                for t in range(NT):
                    for h in range(2):
                        nc.tensor.matmul(
                            out=pe[:, t : t + 1],
                            lhsT=hsT[:, h * S + t * 128 : h * S + (t + 1) * 128],
                            rhs=va[:, h : h + 1],
                            start=(h == 0),
                            stop=(h == 1),
                        )

                # ---- epilogue: lrelu(energy + ca) ----
                esb = small.tile([128, NT], f32, tag="esb")
                nc.scalar.activation(
                    out=esb[:],
                    in_=pe[:],
                    func=AF.Lrelu,
                    bias=ax[:, NT : NT + 1],
                    scale=1.0,
                    alpha=NEG_SLOPE,
                )
                # ---- mask via min(e, cap), then row-max ----
                em = small.tile([128, NT], f32, tag="em")
                nc.vector.tensor_tensor(out=em[:], in0=esb[:], in1=ax[:, 0:NT], op=ALU.min)
                mrow = small.tile([128, 1], f32, tag="mrow")
                nc.vector.reduce_max(out=mrow[:], in_=em[:], axis=mybir.AxisListType.X)
                # ---- global max: transpose mrow via identity-matmul, reduce,
                #      broadcast back via ones-matmul, negate ----
                psT = pss.tile([1, 128], f32, tag="ps_small")
                nc.tensor.matmul(out=psT[:], lhsT=mrow[:], rhs=ident, start=True, stop=True)
                msc = small.tile([1, 1], f32, tag="msc")
                nc.vector.reduce_max(out=msc[:], in_=psT[:], axis=mybir.AxisListType.X)
                psB = pss.tile([128, 1], f32, tag="ps_small")
                nc.tensor.matmul(out=psB[:], lhsT=onesrow, rhs=msc[:], start=True, stop=True)
                negm = small.tile([128, 1], f32, tag="negm")
                nc.scalar.activation(out=negm[:], in_=psB[:], func=AF.Copy, scale=-1.0)
                # ---- exp (+ row sums) ----
                p16 = small.tile([128, NT], f16, tag="p16")
                zrow = small.tile([128, 1], f32, tag="zrow")
                nc.scalar.activation(
                    out=p16[:],
                    in_=em[:],
                    func=AF.Exp,
                    bias=negm[:],
                    scale=1.0,
                    accum_out=zrow[:],
                )
                # ---- global sum via ones-matmul, reciprocal, broadcast ----
                psZ = pss.tile([1, 1], f32, tag="ps_small")
                nc.tensor.matmul(out=psZ[:], lhsT=zrow[:], rhs=onescol, start=True, stop=True)
                rz1 = small.tile([1, 1], f32, tag="rz1")
                nc.vector.reciprocal(rz1[:], psZ[:])
                psR = pss.tile([128, 1], f32, tag="ps_small")
                nc.tensor.matmul(out=psR[:], lhsT=onesrow, rhs=rz1[:], start=True, stop=True)
                rz = small.tile([128, 1], f32, tag="rz")
                nc.scalar.activation(out=rz[:], in_=psR[:], func=AF.Copy, scale=1.0)

                # ---- alpha output ----
                a32 = small.tile([128, NT], f32, tag="a32")
                nc.vector.tensor_scalar_mul(a32[:], p16[:], rz[:])
                nc.sync.dma_start(out=alpha_o[b], in_=a32[:])

                # ---- u = sum_s p[s]*hs[s,:] (PE, accumulated), then /z ----
                pu = psu.tile([1, E], f32, tag="pu")
                for t in range(NT):
                    nc.tensor.matmul(
                        out=pu[:],
                        lhsT=p16[:, t : t + 1],
                        rhs=hsn[:, t, :],
                        start=(t == 0),
                        stop=(t == NT - 1),
                    )
                usb = small.tile([1, E], f32, tag="usb")
                nc.scalar.activation(
                    out=usb[:], in_=pu[:], func=AF.Copy, scale=rz[0:1, :]
                )
                nc.sync.dma_start(out=u_o[b], in_=usb[:])
    _split_sync_waits(nc)
    return nc


_NC_CACHE = None
last_results = None
last_in_maps = None  # exposes BassKernelResults (exec_time_ns etc.) to test.py


def kernel(hs, ht, state_len, Wa_w, Wa_b, Wc_w, Wc_b, **run_kwargs):
    global _NC_CACHE, last_results, last_in_maps
    hs = np.asarray(hs, np.float32)
    ht = np.asarray(ht, np.float32)
    state_len = np.asarray(state_len, np.int32)
    Wa_w = np.asarray(Wa_w, np.float32)
    Wa_b = np.asarray(Wa_b, np.float32)
    Wc_w = np.asarray(Wc_w, np.float32)
    Wc_b = np.asarray(Wc_b, np.float32)

    # host-side tiny precomputes
    va = ht @ Wa_w  # [B, E]
    ca = ht @ Wa_b  # [B]
    hs16 = hs.astype(np.float16)
    hsT16 = np.ascontiguousarray(hs16.transpose(0, 2, 1))  # [B, E, S]
    va16_dev = np.ascontiguousarray(
        va.astype(np.float16).reshape(B, 2, 128).transpose(0, 2, 1)
    )  # [B, 128, 2]; va16_dev[b, p, h] = va[b, h*128+p]
    pos = np.arange(S, dtype=np.int64)
    valid = pos[None, :] < state_len[:, None].astype(np.int64)  # [B, S]
    cap = np.where(valid, 3.0e38, MASK_VAL).astype(np.float32)  # [B, S]
    cap_dev = cap.reshape(B, NT, 128).transpose(0, 2, 1)  # [B, 128, NT]
    aux = np.concatenate(
        [
            cap_dev,
            np.broadcast_to(ca.astype(np.float32)[:, None, None], (B, 128, 1)),
        ],
        axis=2,
    )
    aux = np.ascontiguousarray(aux, np.float32)
    consts = np.concatenate(
        [np.eye(128, dtype=np.float32), np.ones((128, 128), np.float32)], axis=1
    )

    if _NC_CACHE is None:
        _NC_CACHE = _build_bass()
    nc = _NC_CACHE

    in_maps = []
    for i in range(NCORES):
        sl = slice(i * BL, (i + 1) * BL)
        in_maps.append(
            {
                "hs16": hs16[sl],
                "hsT16": hsT16[sl],
                "va16": va16_dev[sl],
                "aux": aux[sl],
                "consts": consts,
            }
        )
    last_in_maps = in_maps
    res = run_bass_kernel_spmd(nc, in_maps, core_ids=list(range(NCORES)), **run_kwargs)
    last_results = res

    alpha_dev = np.stack([res.results[i]["alpha"] for i in range(NCORES)])
    # alpha_dev[core, b, p, t] -> s = t*128 + p
    alpha = (
        alpha_dev.transpose(0, 1, 3, 2).reshape(B, S).astype(np.float32)
    )
    u = np.concatenate(
        [res.results[i]["u"].reshape(BL, E) for i in range(NCORES)]
    )  # [B, E]
    context = (u @ Wc_w.T + Wc_b).astype(np.float32)
    return alpha, context
